# revision 10
# baseline (speedup 1.0000x reference)
"""Trainium2 Bass kernel for BoundaryPointTransformer (gnn_message_passing).

v2 design (8 NeuronCores, data-parallel over points, full table per core):
  - Table rows [NPAD, 136] f32: [k_tilde 0:64 | pW_j 64:67 | 0 | V 68:132 | pad].
    pW = W1s3 @ p folded into the table so the pair 3x3 transform disappears.
  - Phase A builds the table point-major directly (fp16 matmuls, out [128,136]
    per chunk, DMA straight from PSUM) - no transposes, no PSUM->SBUF copies.
  - Phase B gathers 1024 rows per 64-pt block in ONE indirect DMA (descriptor
    batching; the old 8-per-block version was SWDGE-overhead bound).
  - Logit path: one [128,68]-col transpose per chunk covers k+pW; q/pW_i
    expansion via a single E2 matmul (lhsT = [-q | -pW_i] rows of the q-table).
  - Value path is done with PE matmuls against block-diagonal 0/1 constants
    (E16) instead of DVE mult+strided-reduce: num/den/R accumulate point-major
    in PSUM, so no softmax-weight broadcast DMA and no output transpose.
"""

import numpy as np

import sys

sys.path.insert(0, "/opt/trn_rl_repo")

import concourse.bass as bass
import concourse.bacc as bacc
import concourse.mybir as mybir
import concourse.tile as tile
from concourse.bass import AP, IndirectOffsetOnAxis

F32 = mybir.dt.float32
F32R = mybir.dt.float32r
F16 = mybir.dt.float16
BF16 = mybir.dt.bfloat16
I32 = mybir.dt.int32
EPS = 1e-5

N = 100000
C = 64
NS = 16
S = 8

NCORES = 8
NPC = N // NCORES            # 12500 points per core
PTS_BLK = 64
PAIRS_BLK = PTS_BLK * NS     # 1024
NBLK = -(-NPC // PTS_BLK)    # 196
NPP = NBLK * PTS_BLK         # 12544
NCHUNK = PAIRS_BLK // 128    # 8
NSC = 2
SCW = PAIRS_BLK // NSC       # 512

ROW = 136                    # table row floats
SUPER = 768                  # A2 points per super-group
NSUP = -(-N // SUPER)        # 131
NPAD = NSUP * SUPER          # 100608
NQC = NPP // 128             # 98 q-table chunks

_CACHED = {}


def _fold_weights(inp):
    f = {}
    s1 = inp["w_bn1_g"] / np.sqrt(inp["w_bn1_v"] + EPS)
    c1 = inp["w_bn1_b"] - inp["w_bn1_m"] * s1
    s2 = inp["w_bn2_g"] / np.sqrt(inp["w_bn2_v"] + EPS)
    c2 = inp["w_bn2_b"] - inp["w_bn2_m"] * s2
    s3 = inp["p_bn_g"] / np.sqrt(inp["p_bn_v"] + EPS)
    c3 = inp["p_bn_b"] - inp["p_bn_m"] * s3

    Wk1 = s1[:, None] * inp["Wk"]          # (64, 64)
    Wq1 = s1[:, None] * inp["Wq"]          # (64, 64)
    W1s3 = s3[:, None] * inp["p_w1"]       # (3, 3)

    # table build: row_pt = xpt_col @ Wtab ; xpt rows = [x 64 | p 3 | 1]
    Wtab = np.zeros((68, ROW), np.float32)
    Wtab[:64, :64] = Wk1.T
    Wtab[64:67, 64:67] = W1s3.T
    Wtab[:64, 68:132] = inp["Wv"].T
    Wtab[67, 68:132] = inp["bv"] + inp["p_b2"]
    f["Wtab"] = Wtab.astype(np.float16)

    # q-table: row_pt = [-q_tilde 64 | -pW_i 3 | 0]
    Wq68 = np.zeros((68, 68), np.float32)
    Wq68[:64, :64] = -Wq1.T
    Wq68[64:67, 64:67] = -W1s3.T
    f["Wq68"] = Wq68.astype(np.float16)

    f["W2L"] = np.ascontiguousarray((s1[:, None] * inp["p_w2"]).T)  # (3, 64)
    W1p = s2[:, None] * inp["w_w1"]        # (8, 64)
    f["W1pT2"] = np.tile(W1p.T, (2, 1)).astype(np.float32)  # (128, 8)
    f["W2T"] = np.ascontiguousarray(inp["w_w2"].T)          # (8, 8)

    # value-side p_w2 expanded over share-groups: W2VE[(d,t), c] = p_w2[c,d]*[t==c%8]
    W2VE = np.zeros((24, 64), np.float32)
    for d in range(3):
        for c in range(64):
            W2VE[d * 8 + c % 8, c] = inp["p_w2"][c, d]
    f["W2VE"] = W2VE

    f["biasU"] = (s1 * (inp["bk"] - inp["bq"] + inp["p_b2"]) + c1).astype(
        np.float32
    ).reshape(64, 1)
    bias3 = np.zeros((4, 1), np.float32)
    bias3[:3, 0] = s3 * inp["p_b1"] + c3
    f["bias3"] = bias3
    f["bias1p"] = (s2 * inp["w_b1"] + c2).astype(np.float32).reshape(8, 1)
    f["bias2p"] = inp["w_b2"].astype(np.float32).reshape(8, 1)
    return f


def _host_prep(inp):
    import ml_dtypes

    f = _fold_weights(inp)
    x = np.asarray(inp["x"], np.float32)
    p = np.asarray(inp["p"], np.float32)
    idx = np.asarray(inp["idx"]).astype(np.int32)

    xpt = np.zeros((68, NPAD), np.float32)
    xpt[:64, :N] = x.T
    xpt[64:67, :N] = p.T
    xpt[67, :N] = 1.0
    xpt16 = xpt.astype(np.float16)

    # E2[sc*64 + pt, p] = 1 if (sc*512 + p) // 16 == pt
    E2 = np.zeros((128, SCW), np.float32)
    for sc in range(NSC):
        pair_pt = (np.arange(SCW) + sc * SCW) // NS
        E2[sc * 64:sc * 64 + 64] = (
            np.arange(PTS_BLK)[:, None] == pair_pt[None, :]
        )
    # E16[p, m] = 1 if p//16 == m  (within a 128-pair chunk)
    E16 = (np.arange(128)[:, None] // 16 == np.arange(8)[None, :])
    # E64[p, k*64 + m] = 1 if m == k*8 + p//16 (num/den accumulate at base 0)
    E64 = np.zeros((128, 8 * 64), np.float32)
    for k in range(8):
        E64[:, k * 64:(k + 1) * 64] = (
            np.arange(128)[:, None] // 16 + k * 8 == np.arange(64)[None, :]
        )

    ident = np.eye(128, dtype=np.float32)

    E2X = np.vstack([E2[64:], E2[:64]])
    shared = dict(
        xpt=xpt16, E2=np.ascontiguousarray(E2),
        E2b=np.ascontiguousarray(E2X),
        E16=E16.astype(ml_dtypes.bfloat16),
        E64=E64.astype(ml_dtypes.bfloat16), ident=ident,
        Wtab=f["Wtab"], Wq68=f["Wq68"], W2L=f["W2L"], W1pT2=f["W1pT2"],
        W2T=f["W2T"], W2VE=f["W2VE"].astype(ml_dtypes.bfloat16),
        biasU=f["biasU"], bias3=f["bias3"], bias1p=f["bias1p"],
        bias2p=f["bias2p"],
    )

    per_core = []
    for c in range(NCORES):
        lo = c * NPC
        idx_loc = np.zeros((NPP, NS), np.int32)
        idx_loc[:NPC] = idx[lo:lo + NPC]
        flat = idx_loc.reshape(-1)
        idxT = np.ascontiguousarray(
            flat.reshape(NBLK, NCHUNK, 128).transpose(2, 0, 1).reshape(128, -1)
        )
        xpt_loc = np.zeros((68, NPP), np.float16)
        hi = min(lo + NPP, N)
        xpt_loc[:, : hi - lo] = xpt16[:, lo:hi]
        per_core.append(dict(idxT=idxT, xpt_loc=xpt_loc))
    return shared, per_core


def _r(ap):
    return ap.bitcast(F32R)


def _build_program():
    nc = bacc.Bacc("TRN2", target_bir_lowering=False, debug=False)

    d_xpt = nc.dram_tensor("xpt", [68, NPAD], F16, kind="ExternalInput")
    d_xpt_loc = nc.dram_tensor("xpt_loc", [68, NPP], F16, kind="ExternalInput")
    d_E2 = nc.dram_tensor("E2", [128, SCW], F32, kind="ExternalInput")
    d_E2b = nc.dram_tensor("E2b", [128, SCW], F32, kind="ExternalInput")
    d_E16 = nc.dram_tensor("E16", [128, 8], BF16, kind="ExternalInput")
    d_E64 = nc.dram_tensor("E64", [128, 512], BF16, kind="ExternalInput")
    d_ident = nc.dram_tensor("ident", [128, 128], F32, kind="ExternalInput")
    d_Wtab = nc.dram_tensor("Wtab", [68, ROW], F16, kind="ExternalInput")
    d_Wq68 = nc.dram_tensor("Wq68", [68, 68], F16, kind="ExternalInput")
    d_W2L = nc.dram_tensor("W2L", [3, 64], F32, kind="ExternalInput")
    d_W1pT2 = nc.dram_tensor("W1pT2", [128, 8], F32, kind="ExternalInput")
    d_W2T = nc.dram_tensor("W2T", [8, 8], F32, kind="ExternalInput")
    d_W2VE = nc.dram_tensor("W2VE", [24, 64], BF16, kind="ExternalInput")
    d_biasU = nc.dram_tensor("biasU", [64, 1], F32, kind="ExternalInput")
    d_bias3 = nc.dram_tensor("bias3", [4, 1], F32, kind="ExternalInput")
    d_bias1p = nc.dram_tensor("bias1p", [8, 1], F32, kind="ExternalInput")
    d_bias2p = nc.dram_tensor("bias2p", [8, 1], F32, kind="ExternalInput")
    d_idxT = nc.dram_tensor("idxT", [128, NBLK * NCHUNK], I32, kind="ExternalInput")
    d_out = nc.dram_tensor("out", [NPP, C], F32, kind="ExternalOutput")
    d_tab = nc.dram_tensor("tabKV", [NPAD, ROW], F32, kind="Internal")

    RELU = mybir.ActivationFunctionType.Relu
    EXPF = mybir.ActivationFunctionType.Exp
    MULT = mybir.AluOpType.mult
    ADD = mybir.AluOpType.add
    MAX = mybir.AluOpType.max

    with tile.TileContext(nc) as tc:
        with tc.tile_pool(name="const", bufs=1) as cp:
            def tile_from(dram, dt, name):
                t = cp.tile(list(dram.shape), dt, name=name)
                if dt == F32:
                    nc.sync.dma_start(out=_r(t[:, :]), in_=_r(dram.ap()))
                else:
                    nc.sync.dma_start(out=t[:, :], in_=dram.ap())
                return t

            E2S = tile_from(d_E2, F32, "E2S")
            E2bS = tile_from(d_E2b, F32, "E2bS")
            E16S = tile_from(d_E16, BF16, "E16S")
            E64S = tile_from(d_E64, BF16, "E64S")
            identS = tile_from(d_ident, F32, "identS")
            WtabS = tile_from(d_Wtab, F16, "WtabS")
            Wq68S = tile_from(d_Wq68, F16, "Wq68S")
            W2LS = tile_from(d_W2L, F32, "W2LS")
            W1pT2S = tile_from(d_W1pT2, F32, "W1pT2S")
            W2TS = tile_from(d_W2T, F32, "W2TS")
            W2VES = tile_from(d_W2VE, BF16, "W2VES")
            biasUS = tile_from(d_biasU, F32, "biasUS")
            bias3S = tile_from(d_bias3, F32, "bias3S")
            bias1pS = tile_from(d_bias1p, F32, "bias1pS")
            bias2pS = tile_from(d_bias2p, F32, "bias2pS")
            idxTS = cp.tile_from(d_idxT.ap())
            qtab = cp.tile([128, NQC * 68], F32, name="qtab")

            # ---------------- Phase A1: q-table (point-major, SBUF) --------
            with (
                tc.tile_pool(name="qb", bufs=1) as qb,
                tc.tile_pool(name="qbp", bufs=3, space="PSUM") as qbp,
            ):
                xptL = qb.tile([68, NPP], F16, name="xptL")
                nc.sync.dma_start(out=xptL[:, :], in_=d_xpt_loc.ap())
                for q in range(NQC):
                    Pq = qbp.tile([128, 68], F32, name="Pq")
                    nc.tensor.matmul(
                        out=Pq[:, :],
                        lhsT=xptL[:, q * 128:(q + 1) * 128],
                        rhs=Wq68S[:, :],
                        start=True, stop=True,
                    )
                    dst = _r(qtab[:, q * 68:(q + 1) * 68])
                    if q % 2 == 0:
                        nc.scalar.copy(out=dst, in_=Pq[:, :])
                    else:
                        nc.vector.tensor_copy(out=dst, in_=Pq[:, :])

            # ---------------- Phase A2: tabKV build (point-major) ----------
            with (
                tc.tile_pool(name="tb", bufs=2) as tb,
                tc.tile_pool(name="tbp", bufs=4, space="PSUM") as tbp,
            ):
                for g in range(NSUP):
                    xg = tb.tile([68, SUPER], F16, name="xg")
                    nc.sync.dma_start(
                        out=xg[:, :],
                        in_=d_xpt.ap()[:, g * SUPER:(g + 1) * SUPER],
                    )
                    for h in range(2):
                        Pt = tbp.tile([128, 3 * ROW], F32, name="Pt", tag="Pt")
                        for j in range(3):
                            nc.tensor.matmul(
                                out=Pt[:, j * ROW:(j + 1) * ROW],
                                lhsT=xg[:, (h * 3 + j) * 128:(h * 3 + j + 1) * 128],
                                rhs=WtabS[:, :],
                                start=True, stop=True,
                                skip_group_check=True,
                            )
                        cS = tb.tile([128, 3 * ROW], F32, name="cS", tag="cS")
                        if h == 0:
                            nc.scalar.copy(out=_r(cS[:, :]), in_=Pt[:, :])
                        else:
                            nc.vector.tensor_copy(out=_r(cS[:, :]), in_=Pt[:, :])
                        r0 = g * SUPER + h * 384
                        dram_ap = AP(
                            d_tab.ap().tensor, r0 * ROW,
                            [[ROW, 128], [ROW * 128, 3], [1, ROW]],
                        )
                        src_ap = AP(
                            cS.tensor, cS.offset,
                            [[3 * ROW, 128], [ROW, 3], [1, ROW]],
                        )
                        if h == 0:
                            nc.sync.dma_start(out=dram_ap, in_=src_ap)
                        else:
                            nc.scalar.dma_start(out=dram_ap, in_=src_ap)

            # ---------------- Phase B: main loop ----------------
            with (
                tc.tile_pool(name="mw", bufs=2) as mw,
                tc.tile_pool(name="mw2", bufs=2) as mw2,
                tc.tile_pool(name="pkv", bufs=2, space="PSUM") as pkv_pool,
                tc.tile_pool(name="psA", bufs=2, space="PSUM") as psA,
            ):
                for b in range(NBLK):
                    G = mw.tile([128, NCHUNK * ROW], F32, name="G")
                    nc.gpsimd.indirect_dma_start(
                        out=_r(G[:, :]), out_offset=None,
                        in_=_r(d_tab.ap()),
                        in_offset=IndirectOffsetOnAxis(
                            ap=idxTS[:, b * NCHUNK:(b + 1) * NCHUNK], axis=0
                        ),
                    )

                    # transposes: [k | pW] columns -> channel-major PSUM
                    Pkv = pkv_pool.tile([68, PAIRS_BLK], F32, name="Pkv")
                    for k in range(NCHUNK):
                        nc.tensor.matmul(
                            out=_r(Pkv[:, k * 128:(k + 1) * 128]),
                            lhsT=_r(G[:, k * ROW:k * ROW + 68]),
                            rhs=_r(identS[:, :]),
                            is_transpose=True, start=(k % 4 == 0), stop=False,
                            skip_group_check=True,
                        )
                    # [-q | -pW_i] expansion
                    qsl = qtab[
                        64 * (b % 2):64 * (b % 2) + 64,
                        (b // 2) * 68:(b // 2) * 68 + 68,
                    ]
                    qb0 = 64 * (b % 2)
                    for sc in range(NSC):
                        e2sc = E2S if sc == b % 2 else E2bS
                        nc.tensor.matmul(
                            out=Pkv[:, sc * SCW:(sc + 1) * SCW],
                            lhsT=_r(qsl), rhs=_r(e2sc[qb0:qb0 + 64, :]),
                            start=False, stop=False, skip_group_check=True,
                        )
                    # r3 = relu(pW_j - pW_i + bias3)
                    r3S = mw2.tile([4, PAIRS_BLK], F32, name="r3S")
                    nc.scalar.activation(
                        out=_r(r3S[:, :]), in_=Pkv[64:68, :], func=RELU,
                        bias=bias3S[:, :],
                    )
                    # logit-side p_r: accumulate W2L @ r3 into Pkv rows 0:64
                    for sc in range(NSC):
                        nc.tensor.matmul(
                            out=Pkv[0:64, sc * SCW:(sc + 1) * SCW],
                            lhsT=_r(W2LS[:, :]),
                            rhs=_r(r3S[0:3, sc * SCW:(sc + 1) * SCW]),
                            start=False, stop=True, skip_group_check=True,
                        )

                    # u2 = relu(logits + biasU), [sc*64+ch, p]
                    u2 = mw.tile([128, SCW], F32, name="u2")
                    nc.scalar.activation(
                        out=_r(u2[0:64, :]), in_=Pkv[0:64, 0:SCW],
                        func=RELU, bias=biasUS[:, :],
                    )
                    nc.vector.tensor_scalar(
                        out=_r(u2[64:128, :]), in0=Pkv[0:64, SCW:2 * SCW],
                        scalar1=biasUS[:, :], scalar2=0.0,
                        op0=ADD, op1=MAX,
                    )

                    Py1 = psA.tile([8, PAIRS_BLK], F32, name="Py1", tag="psA")
                    for sc in range(NSC):
                        nc.tensor.matmul(
                            out=Py1[:, sc * SCW:(sc + 1) * SCW],
                            lhsT=_r(W1pT2S[64 * sc:64 * sc + 64, :]),
                            rhs=_r(u2[64 * sc:64 * sc + 64, :]),
                            start=True, stop=True, skip_group_check=True,
                        )
                    y1S = mw2.tile([8, PAIRS_BLK], F32, name="y1S")
                    nc.vector.tensor_scalar(
                        out=_r(y1S[:, :]), in0=Py1[:, :],
                        scalar1=bias1pS[:, :], scalar2=0.0,
                        op0=ADD, op1=MAX,
                    )
                    PL = psA.tile([8, PAIRS_BLK], F32, name="PL", tag="psA")
                    for sc in range(NSC):
                        nc.tensor.matmul(
                            out=PL[:, sc * SCW:(sc + 1) * SCW],
                            lhsT=_r(W2TS[:, :]),
                            rhs=_r(y1S[:, sc * SCW:(sc + 1) * SCW]),
                            start=True, stop=True, skip_group_check=True,
                        )
                    eS = mw2.tile([8, PAIRS_BLK], F32, name="eS")
                    nc.scalar.activation(
                        out=_r(eS[:, :]), in_=PL[:, :], func=EXPF,
                        bias=bias2pS[:, :],
                    )

                    # pair-major e and r3 via PE transposes into one PSUM tile
                    epr3P = psA.tile([128, 96], F32, name="epr3P", tag="psA")
                    for k in range(NCHUNK):
                        nc.tensor.matmul(
                            out=_r(epr3P[:, k * 8:k * 8 + 8]),
                            lhsT=_r(eS[:, k * 128:(k + 1) * 128]),
                            rhs=_r(identS[0:8, 0:8]),
                            is_transpose=True, start=True, stop=True,
                            skip_group_check=True,
                        )
                        nc.tensor.matmul(
                            out=_r(epr3P[:, 64 + k * 4:64 + k * 4 + 4]),
                            lhsT=_r(r3S[0:4, k * 128:(k + 1) * 128]),
                            rhs=_r(identS[0:4, 0:4]),
                            is_transpose=True, start=True, stop=True,
                            skip_group_check=True,
                        )
                    # e pair-major in f32 (for mults)
                    e_pmS = mw2.tile([128, 64], F32, name="e_pmS")
                    nc.scalar.copy(out=e_pmS[:, :], in_=epr3P[:, 0:64])

                    # uniS cols per chunk k: [vw 64 | e 8 | er3 24], bf16
                    uniS = mw.tile([128, NCHUNK * 96], BF16, name="uniS")
                    with nc.allow_low_precision(reason="bf16 staging"):
                        # e columns (for den matmul)
                        nc.scalar.copy(
                            out=AP(uniS.tensor, uniS.offset + 64,
                                   [[NCHUNK * 96, 128], [96, NCHUNK], [1, 8]]),
                            in_=epr3P[:, 0:64],
                        )
                        # vw = V * e[t]: 4 ops on gpsimd, 4 on vector
                        for s in range(8):
                            outap = AP(uniS.tensor, uniS.offset + s * 8,
                                       [[NCHUNK * 96, 128], [96, NCHUNK], [1, 8]])
                            in0 = AP(G.tensor, G.offset + 68 + s * 8,
                                     [[NCHUNK * ROW, 128], [ROW, NCHUNK], [1, 8]])
                            in1 = AP(e_pmS.tensor, e_pmS.offset,
                                     [[64, 128], [8, NCHUNK], [1, 8]])
                            eng = nc.gpsimd if s % 2 == 0 else nc.vector
                            eng.tensor_tensor(out=outap, in0=in0, in1=in1, op=MULT)
                        # er3[(d,t)] = r3_pm[d] * e[t]
                        for d in range(3):
                            outap = AP(uniS.tensor, uniS.offset + 72 + d * 8,
                                       [[NCHUNK * 96, 128], [96, NCHUNK], [1, 8]])
                            in0 = AP(epr3P.tensor, epr3P.offset + 64 + d,
                                     [[96, 128], [4, NCHUNK], [0, 8]])
                            in1 = AP(e_pmS.tensor, e_pmS.offset,
                                     [[64, 128], [8, NCHUNK], [1, 8]])
                            nc.vector.tensor_tensor(
                                out=outap, in0=in0, in1=in1, op=MULT
                            )

                    # num/den and R via block-diagonal matmuls
                    ndP = psA.tile([64, 72], F32, name="ndP", tag="psA")
                    RP = psA.tile([24, 64], F32, name="RP", tag="psA")
                    for k in range(NCHUNK):
                        nc.tensor.matmul(
                            out=ndP[:, :],
                            lhsT=E64S[:, k * 64:(k + 1) * 64],
                            rhs=uniS[:, k * 96:k * 96 + 72],
                            start=(k == 0), stop=False, skip_group_check=True,
                        )
                        nc.tensor.matmul(
                            out=RP[:, k * 8:k * 8 + 8],
                            lhsT=uniS[:, k * 96 + 72:k * 96 + 96],
                            rhs=E16S[:, :],
                            start=True, stop=True, skip_group_check=True,
                        )
                    RS = mw2.tile([24, 64], BF16, name="RS")
                    with nc.allow_low_precision(reason="bf16 staging"):
                        nc.scalar.copy(out=RS[:, :], in_=RP[:, :])
                    # value-side p_r contribution, accumulated into num
                    nc.tensor.matmul(
                        out=ndP[:, 0:64],
                        lhsT=RS[:, :], rhs=W2VES[:, :],
                        start=False, stop=True, skip_group_check=True,
                    )

                    recipS = mw2.tile([64, 8], F32, name="recipS")
                    with nc.allow_low_precision(reason="f32r bitcast, same width"):
                        nc.vector.reciprocal(
                            out=_r(recipS[:, :]), in_=ndP[:, 64:72]
                        )
                    if b % 2 == 0:
                        outS2 = mw2.tile([64, 128], F32, name="outS2", tag="o2")
                    nc.vector.tensor_tensor(
                        out=_r(outS2[:, (b % 2) * 64:(b % 2) * 64 + 64]),
                        in0=ndP[:, 0:64],
                        in1=AP(recipS.tensor, recipS.offset,
                               [[8, 64], [0, 8], [1, 8]]),
                        op=MULT,
                    )
                    if b % 2 == 1:
                        dst = AP(
                            d_out.ap().tensor, (b - 1) * PTS_BLK * C,
                            [[C, 64], [C * 64, 2], [1, C]],
                        )
                        src = AP(
                            outS2.tensor, outS2.offset,
                            [[128, 64], [64, 2], [1, 64]],
                        )
                        nc.sync.dma_start(out=dst, in_=src)

    nc.compile()
    return nc


def kernel(**inputs):
    from concourse.bass_utils import run_bass_kernel_spmd

    shared, per_core = _host_prep(inputs)

    if "nc" not in _CACHED:
        _CACHED["nc"] = _build_program()
    nc = _CACHED["nc"]

    in_maps = []
    for c in range(NCORES):
        m = dict(shared)
        m.update(per_core[c])
        in_maps.append(m)

    res = run_bass_kernel_spmd(nc, in_maps, core_ids=list(range(NCORES)))
    out = np.empty((N, C), np.float32)
    for c in range(NCORES):
        out[c * NPC:(c + 1) * NPC] = res.results[c]["out"][:NPC]
    return out


# revision 11
# speedup vs baseline: 22.7415x; 22.7415x over previous
"""Trainium2 Bass kernel for BoundaryPointTransformer (gnn_message_passing).

v2 design (8 NeuronCores, data-parallel over points, full table per core):
  - Table rows [NPAD, 136] f32: [k_tilde 0:64 | pW_j 64:67 | 0 | V 68:132 | pad].
    pW = W1s3 @ p folded into the table so the pair 3x3 transform disappears.
  - Phase A builds the table point-major directly (fp16 matmuls, out [128,136]
    per chunk, DMA straight from PSUM) - no transposes, no PSUM->SBUF copies.
  - Phase B gathers 1024 rows per 64-pt block in ONE indirect DMA (descriptor
    batching; the old 8-per-block version was SWDGE-overhead bound).
  - Logit path: one [128,68]-col transpose per chunk covers k+pW; q/pW_i
    expansion via a single E2 matmul (lhsT = [-q | -pW_i] rows of the q-table).
  - Value path is done with PE matmuls against block-diagonal 0/1 constants
    (E16) instead of DVE mult+strided-reduce: num/den/R accumulate point-major
    in PSUM, so no softmax-weight broadcast DMA and no output transpose.
"""

import numpy as np

import sys

sys.path.insert(0, "/opt/trn_rl_repo")

import concourse.bass as bass
import concourse.bacc as bacc
import concourse.mybir as mybir
import concourse.tile as tile
from concourse.bass import AP, IndirectOffsetOnAxis

F32 = mybir.dt.float32
F32R = mybir.dt.float32r
F16 = mybir.dt.float16
BF16 = mybir.dt.bfloat16
I32 = mybir.dt.int32
EPS = 1e-5

N = 100000
C = 64
NS = 16
S = 8

NCORES = 8
NPC = N // NCORES            # 12500 points per core
PTS_BLK = 64
PAIRS_BLK = PTS_BLK * NS     # 1024
NBLK = -(-NPC // PTS_BLK)    # 196
NPP = NBLK * PTS_BLK         # 12544
NCHUNK = PAIRS_BLK // 128    # 8
NSC = 2
SCW = PAIRS_BLK // NSC       # 512

ROW = 136                    # table row floats
SUPER = 768                  # A2 points per super-group
NSUP = -(-N // SUPER)        # 131
NPAD = NSUP * SUPER          # 100608
NQC = NPP // 128             # 98 q-table chunks

_CACHED = {}


def _fold_weights(inp):
    f = {}
    s1 = inp["w_bn1_g"] / np.sqrt(inp["w_bn1_v"] + EPS)
    c1 = inp["w_bn1_b"] - inp["w_bn1_m"] * s1
    s2 = inp["w_bn2_g"] / np.sqrt(inp["w_bn2_v"] + EPS)
    c2 = inp["w_bn2_b"] - inp["w_bn2_m"] * s2
    s3 = inp["p_bn_g"] / np.sqrt(inp["p_bn_v"] + EPS)
    c3 = inp["p_bn_b"] - inp["p_bn_m"] * s3

    Wk1 = s1[:, None] * inp["Wk"]          # (64, 64)
    Wq1 = s1[:, None] * inp["Wq"]          # (64, 64)
    W1s3 = s3[:, None] * inp["p_w1"]       # (3, 3)

    # table build: row_pt = xpt_col @ Wtab ; xpt rows = [x 64 | p 3 | 1]
    Wtab = np.zeros((68, ROW), np.float32)
    Wtab[:64, :64] = Wk1.T
    Wtab[64:67, 64:67] = W1s3.T
    Wtab[:64, 68:132] = inp["Wv"].T
    Wtab[67, 68:132] = inp["bv"] + inp["p_b2"]
    f["Wtab"] = Wtab.astype(np.float16)

    # q-table: row_pt = [-q_tilde 64 | -pW_i 3 | 0]
    Wq68 = np.zeros((68, 68), np.float32)
    Wq68[:64, :64] = -Wq1.T
    Wq68[64:67, 64:67] = -W1s3.T
    f["Wq68"] = Wq68.astype(np.float16)

    f["W2L"] = np.ascontiguousarray((s1[:, None] * inp["p_w2"]).T)  # (3, 64)
    W1p = s2[:, None] * inp["w_w1"]        # (8, 64)
    f["W1pT2"] = np.tile(W1p.T, (2, 1)).astype(np.float32)  # (128, 8)
    f["W2T"] = np.ascontiguousarray(inp["w_w2"].T)          # (8, 8)

    # value-side p_w2 expanded over share-groups: W2VE[(d,t), c] = p_w2[c,d]*[t==c%8]
    W2VE = np.zeros((24, 64), np.float32)
    for d in range(3):
        for c in range(64):
            W2VE[d * 8 + c % 8, c] = inp["p_w2"][c, d]
    f["W2VE"] = W2VE

    f["biasU"] = (s1 * (inp["bk"] - inp["bq"] + inp["p_b2"]) + c1).astype(
        np.float32
    ).reshape(64, 1)
    bias3 = np.zeros((4, 1), np.float32)
    bias3[:3, 0] = s3 * inp["p_b1"] + c3
    f["bias3"] = bias3
    f["bias1p"] = (s2 * inp["w_b1"] + c2).astype(np.float32).reshape(8, 1)
    f["bias2p"] = inp["w_b2"].astype(np.float32).reshape(8, 1)
    return f


def _host_prep(inp):
    import ml_dtypes

    f = _fold_weights(inp)
    x = np.asarray(inp["x"], np.float32)
    p = np.asarray(inp["p"], np.float32)
    idx = np.asarray(inp["idx"]).astype(np.int32)

    xpt = np.zeros((68, NPAD), np.float32)
    xpt[:64, :N] = x.T
    xpt[64:67, :N] = p.T
    xpt[67, :N] = 1.0
    xpt16 = xpt.astype(np.float16)

    # E2[sc*64 + pt, p] = 1 if (sc*512 + p) // 16 == pt
    E2 = np.zeros((128, SCW), np.float32)
    for sc in range(NSC):
        pair_pt = (np.arange(SCW) + sc * SCW) // NS
        E2[sc * 64:sc * 64 + 64] = (
            np.arange(PTS_BLK)[:, None] == pair_pt[None, :]
        )
    # E16[p, m] = 1 if p//16 == m  (within a 128-pair chunk)
    E16 = (np.arange(128)[:, None] // 16 == np.arange(8)[None, :])
    # E64[p, k*64 + m] = 1 if m == k*8 + p//16 (num/den accumulate at base 0)
    E64 = np.zeros((128, 8 * 64), np.float32)
    for k in range(8):
        E64[:, k * 64:(k + 1) * 64] = (
            np.arange(128)[:, None] // 16 + k * 8 == np.arange(64)[None, :]
        )

    ident = np.eye(128, dtype=np.float32)

    E2X = np.vstack([E2[64:], E2[:64]])
    shared = dict(
        xpt=xpt16, E2=np.ascontiguousarray(E2),
        E2b=np.ascontiguousarray(E2X),
        E16=E16.astype(ml_dtypes.bfloat16),
        E64=E64.astype(ml_dtypes.bfloat16), ident=ident,
        Wtab=f["Wtab"], Wq68=f["Wq68"], W2L=f["W2L"], W1pT2=f["W1pT2"],
        W2T=f["W2T"], W2VE=f["W2VE"].astype(ml_dtypes.bfloat16),
        biasU=f["biasU"], bias3=f["bias3"], bias1p=f["bias1p"],
        bias2p=f["bias2p"],
    )

    per_core = []
    for c in range(NCORES):
        lo = c * NPC
        idx_loc = np.zeros((NPP, NS), np.int32)
        idx_loc[:NPC] = idx[lo:lo + NPC]
        flat = idx_loc.reshape(-1)
        idxT = np.ascontiguousarray(
            flat.reshape(NBLK, NCHUNK, 128).transpose(2, 0, 1).reshape(128, -1)
        )
        xpt_loc = np.zeros((68, NPP), np.float16)
        hi = min(lo + NPP, N)
        xpt_loc[:, : hi - lo] = xpt16[:, lo:hi]
        per_core.append(dict(idxT=idxT, xpt_loc=xpt_loc))
    return shared, per_core


def _r(ap):
    return ap.bitcast(F32R)


def _build_program():
    nc = bacc.Bacc("TRN2", target_bir_lowering=False, debug=False)

    d_xpt = nc.dram_tensor("xpt", [68, NPAD], F16, kind="ExternalInput")
    d_xpt_loc = nc.dram_tensor("xpt_loc", [68, NPP], F16, kind="ExternalInput")
    d_E2 = nc.dram_tensor("E2", [128, SCW], F32, kind="ExternalInput")
    d_E2b = nc.dram_tensor("E2b", [128, SCW], F32, kind="ExternalInput")
    d_E16 = nc.dram_tensor("E16", [128, 8], BF16, kind="ExternalInput")
    d_E64 = nc.dram_tensor("E64", [128, 512], BF16, kind="ExternalInput")
    d_ident = nc.dram_tensor("ident", [128, 128], F32, kind="ExternalInput")
    d_Wtab = nc.dram_tensor("Wtab", [68, ROW], F16, kind="ExternalInput")
    d_Wq68 = nc.dram_tensor("Wq68", [68, 68], F16, kind="ExternalInput")
    d_W2L = nc.dram_tensor("W2L", [3, 64], F32, kind="ExternalInput")
    d_W1pT2 = nc.dram_tensor("W1pT2", [128, 8], F32, kind="ExternalInput")
    d_W2T = nc.dram_tensor("W2T", [8, 8], F32, kind="ExternalInput")
    d_W2VE = nc.dram_tensor("W2VE", [24, 64], BF16, kind="ExternalInput")
    d_biasU = nc.dram_tensor("biasU", [64, 1], F32, kind="ExternalInput")
    d_bias3 = nc.dram_tensor("bias3", [4, 1], F32, kind="ExternalInput")
    d_bias1p = nc.dram_tensor("bias1p", [8, 1], F32, kind="ExternalInput")
    d_bias2p = nc.dram_tensor("bias2p", [8, 1], F32, kind="ExternalInput")
    d_idxT = nc.dram_tensor("idxT", [128, NBLK * NCHUNK], I32, kind="ExternalInput")
    d_out = nc.dram_tensor("out", [NPP, C], F32, kind="ExternalOutput")
    d_tab = nc.dram_tensor("tabKV", [NPAD, ROW], F32, kind="Internal")

    RELU = mybir.ActivationFunctionType.Relu
    EXPF = mybir.ActivationFunctionType.Exp
    MULT = mybir.AluOpType.mult
    ADD = mybir.AluOpType.add
    MAX = mybir.AluOpType.max

    with tile.TileContext(nc) as tc:
        with tc.tile_pool(name="const", bufs=1) as cp:
            def tile_from(dram, dt, name):
                t = cp.tile(list(dram.shape), dt, name=name)
                if dt == F32:
                    nc.sync.dma_start(out=_r(t[:, :]), in_=_r(dram.ap()))
                else:
                    nc.sync.dma_start(out=t[:, :], in_=dram.ap())
                return t

            E2S = tile_from(d_E2, F32, "E2S")
            E2bS = tile_from(d_E2b, F32, "E2bS")
            E16S = tile_from(d_E16, BF16, "E16S")
            E64S = tile_from(d_E64, BF16, "E64S")
            identS = tile_from(d_ident, F32, "identS")
            WtabS = tile_from(d_Wtab, F16, "WtabS")
            Wq68S = tile_from(d_Wq68, F16, "Wq68S")
            W2LS = tile_from(d_W2L, F32, "W2LS")
            W1pT2S = tile_from(d_W1pT2, F32, "W1pT2S")
            W2TS = tile_from(d_W2T, F32, "W2TS")
            W2VES = tile_from(d_W2VE, BF16, "W2VES")
            biasUS = tile_from(d_biasU, F32, "biasUS")
            bias3S = tile_from(d_bias3, F32, "bias3S")
            bias1pS = tile_from(d_bias1p, F32, "bias1pS")
            bias2pS = tile_from(d_bias2p, F32, "bias2pS")
            idxTS = cp.tile_from(d_idxT.ap())
            qtab = cp.tile([128, NQC * 68], F32, name="qtab")

            # ---------------- Phase A1: q-table (point-major, SBUF) --------
            with (
                tc.tile_pool(name="qb", bufs=1) as qb,
                tc.tile_pool(name="qbp", bufs=3, space="PSUM") as qbp,
            ):
                xptL = qb.tile([68, NPP], F16, name="xptL")
                nc.sync.dma_start(out=xptL[:, :], in_=d_xpt_loc.ap())
                for q in range(NQC):
                    Pq = qbp.tile([128, 68], F32, name="Pq")
                    nc.tensor.matmul(
                        out=Pq[:, :],
                        lhsT=xptL[:, q * 128:(q + 1) * 128],
                        rhs=Wq68S[:, :],
                        start=True, stop=True,
                    )
                    dst = _r(qtab[:, q * 68:(q + 1) * 68])
                    if q % 2 == 0:
                        nc.scalar.copy(out=dst, in_=Pq[:, :])
                    else:
                        nc.vector.tensor_copy(out=dst, in_=Pq[:, :])

            # ---------------- Phase A2: tabKV build (point-major) ----------
            with (
                tc.tile_pool(name="tb", bufs=2) as tb,
                tc.tile_pool(name="tbp", bufs=4, space="PSUM") as tbp,
            ):
                for g in range(NSUP):
                    xg = tb.tile([68, SUPER], F16, name="xg")
                    nc.sync.dma_start(
                        out=xg[:, :],
                        in_=d_xpt.ap()[:, g * SUPER:(g + 1) * SUPER],
                    )
                    for h in range(2):
                        Pt = tbp.tile([128, 3 * ROW], F32, name="Pt", tag="Pt")
                        for j in range(3):
                            nc.tensor.matmul(
                                out=Pt[:, j * ROW:(j + 1) * ROW],
                                lhsT=xg[:, (h * 3 + j) * 128:(h * 3 + j + 1) * 128],
                                rhs=WtabS[:, :],
                                start=True, stop=True,
                                skip_group_check=True,
                            )
                        cS = tb.tile([128, 3 * ROW], F32, name="cS", tag="cS")
                        if h == 0:
                            nc.scalar.copy(out=_r(cS[:, :]), in_=Pt[:, :])
                        else:
                            nc.vector.tensor_copy(out=_r(cS[:, :]), in_=Pt[:, :])
                        r0 = g * SUPER + h * 384
                        dram_ap = AP(
                            d_tab.ap().tensor, r0 * ROW,
                            [[ROW, 128], [ROW * 128, 3], [1, ROW]],
                        )
                        src_ap = AP(
                            cS.tensor, cS.offset,
                            [[3 * ROW, 128], [ROW, 3], [1, ROW]],
                        )
                        if h == 0:
                            nc.sync.dma_start(out=dram_ap, in_=src_ap)
                        else:
                            nc.scalar.dma_start(out=dram_ap, in_=src_ap)

            # ---------------- Phase B: main loop ----------------
            with (
                tc.tile_pool(name="mw", bufs=2) as mw,
                tc.tile_pool(name="mw2", bufs=2) as mw2,
                tc.tile_pool(name="pkv", bufs=2, space="PSUM") as pkv_pool,
                tc.tile_pool(name="psA", bufs=2, space="PSUM") as psA,
            ):
                for b in range(NBLK):
                    G = mw.tile([128, NCHUNK * ROW], F32, name="G")
                    import os as _os
                    if _os.environ.get("K_GATHER_SPLIT"):
                        for k in range(NCHUNK):
                            nc.gpsimd.indirect_dma_start(
                                out=_r(G[:, k * ROW:(k + 1) * ROW]),
                                out_offset=None,
                                in_=_r(d_tab.ap()),
                                in_offset=IndirectOffsetOnAxis(
                                    ap=idxTS[:, b * NCHUNK + k:b * NCHUNK + k + 1],
                                    axis=0,
                                ),
                            )
                    else:
                        nc.gpsimd.indirect_dma_start(
                            out=_r(G[:, :]), out_offset=None,
                            in_=_r(d_tab.ap()),
                            in_offset=IndirectOffsetOnAxis(
                                ap=idxTS[:, b * NCHUNK:(b + 1) * NCHUNK], axis=0
                            ),
                        )

                    # transposes: [k | pW] columns -> channel-major PSUM
                    Pkv = pkv_pool.tile([68, PAIRS_BLK], F32, name="Pkv")
                    for k in range(NCHUNK):
                        nc.tensor.matmul(
                            out=_r(Pkv[:, k * 128:(k + 1) * 128]),
                            lhsT=_r(G[:, k * ROW:k * ROW + 68]),
                            rhs=_r(identS[:, :]),
                            is_transpose=True, start=(k % 4 == 0), stop=False,
                            skip_group_check=True,
                        )
                    # [-q | -pW_i] expansion
                    qsl = qtab[
                        64 * (b % 2):64 * (b % 2) + 64,
                        (b // 2) * 68:(b // 2) * 68 + 68,
                    ]
                    qb0 = 64 * (b % 2)
                    for sc in range(NSC):
                        e2sc = E2S if sc == b % 2 else E2bS
                        nc.tensor.matmul(
                            out=Pkv[:, sc * SCW:(sc + 1) * SCW],
                            lhsT=_r(qsl), rhs=_r(e2sc[qb0:qb0 + 64, :]),
                            start=False, stop=False, skip_group_check=True,
                        )
                    # r3 = relu(pW_j - pW_i + bias3)
                    r3S = mw2.tile([4, PAIRS_BLK], F32, name="r3S")
                    nc.scalar.activation(
                        out=_r(r3S[:, :]), in_=Pkv[64:68, :], func=RELU,
                        bias=bias3S[:, :],
                    )
                    # logit-side p_r: accumulate W2L @ r3 into Pkv rows 0:64
                    for sc in range(NSC):
                        nc.tensor.matmul(
                            out=Pkv[0:64, sc * SCW:(sc + 1) * SCW],
                            lhsT=_r(W2LS[:, :]),
                            rhs=_r(r3S[0:3, sc * SCW:(sc + 1) * SCW]),
                            start=False, stop=True, skip_group_check=True,
                        )

                    # u2 = relu(logits + biasU), [sc*64+ch, p]
                    u2 = mw.tile([128, SCW], F32, name="u2")
                    nc.scalar.activation(
                        out=_r(u2[0:64, :]), in_=Pkv[0:64, 0:SCW],
                        func=RELU, bias=biasUS[:, :],
                    )
                    nc.vector.tensor_scalar(
                        out=_r(u2[64:128, :]), in0=Pkv[0:64, SCW:2 * SCW],
                        scalar1=biasUS[:, :], scalar2=0.0,
                        op0=ADD, op1=MAX,
                    )

                    Py1 = psA.tile([8, PAIRS_BLK], F32, name="Py1", tag="psA")
                    for sc in range(NSC):
                        nc.tensor.matmul(
                            out=Py1[:, sc * SCW:(sc + 1) * SCW],
                            lhsT=_r(W1pT2S[64 * sc:64 * sc + 64, :]),
                            rhs=_r(u2[64 * sc:64 * sc + 64, :]),
                            start=True, stop=True, skip_group_check=True,
                        )
                    y1S = mw2.tile([8, PAIRS_BLK], F32, name="y1S")
                    nc.vector.tensor_scalar(
                        out=_r(y1S[:, :]), in0=Py1[:, :],
                        scalar1=bias1pS[:, :], scalar2=0.0,
                        op0=ADD, op1=MAX,
                    )
                    PL = psA.tile([8, PAIRS_BLK], F32, name="PL", tag="psA")
                    for sc in range(NSC):
                        nc.tensor.matmul(
                            out=PL[:, sc * SCW:(sc + 1) * SCW],
                            lhsT=_r(W2TS[:, :]),
                            rhs=_r(y1S[:, sc * SCW:(sc + 1) * SCW]),
                            start=True, stop=True, skip_group_check=True,
                        )
                    eS = mw2.tile([8, PAIRS_BLK], F32, name="eS")
                    nc.scalar.activation(
                        out=_r(eS[:, :]), in_=PL[:, :], func=EXPF,
                        bias=bias2pS[:, :],
                    )

                    # pair-major e and r3 via PE transposes into one PSUM tile
                    epr3P = psA.tile([128, 96], F32, name="epr3P", tag="psA")
                    for k in range(NCHUNK):
                        nc.tensor.matmul(
                            out=_r(epr3P[:, k * 8:k * 8 + 8]),
                            lhsT=_r(eS[:, k * 128:(k + 1) * 128]),
                            rhs=_r(identS[0:8, 0:8]),
                            is_transpose=True, start=True, stop=True,
                            skip_group_check=True,
                        )
                        nc.tensor.matmul(
                            out=_r(epr3P[:, 64 + k * 4:64 + k * 4 + 4]),
                            lhsT=_r(r3S[0:4, k * 128:(k + 1) * 128]),
                            rhs=_r(identS[0:4, 0:4]),
                            is_transpose=True, start=True, stop=True,
                            skip_group_check=True,
                        )
                    # e pair-major in f32 (for mults)
                    e_pmS = mw2.tile([128, 64], F32, name="e_pmS")
                    nc.scalar.copy(out=e_pmS[:, :], in_=epr3P[:, 0:64])

                    # uniS cols per chunk k: [vw 64 | e 8 | er3 24], bf16
                    uniS = mw.tile([128, NCHUNK * 96], BF16, name="uniS")
                    with nc.allow_low_precision(reason="bf16 staging"):
                        # e columns (for den matmul)
                        nc.scalar.copy(
                            out=AP(uniS.tensor, uniS.offset + 64,
                                   [[NCHUNK * 96, 128], [96, NCHUNK], [1, 8]]),
                            in_=epr3P[:, 0:64],
                        )
                        # vw = V * e[t]: 4 ops on gpsimd, 4 on vector
                        for s in range(8):
                            outap = AP(uniS.tensor, uniS.offset + s * 8,
                                       [[NCHUNK * 96, 128], [96, NCHUNK], [1, 8]])
                            in0 = AP(G.tensor, G.offset + 68 + s * 8,
                                     [[NCHUNK * ROW, 128], [ROW, NCHUNK], [1, 8]])
                            in1 = AP(e_pmS.tensor, e_pmS.offset,
                                     [[64, 128], [8, NCHUNK], [1, 8]])
                            eng = nc.gpsimd if s % 2 == 0 else nc.vector
                            eng.tensor_tensor(out=outap, in0=in0, in1=in1, op=MULT)
                        # er3[(d,t)] = r3_pm[d] * e[t]
                        for d in range(3):
                            outap = AP(uniS.tensor, uniS.offset + 72 + d * 8,
                                       [[NCHUNK * 96, 128], [96, NCHUNK], [1, 8]])
                            in0 = AP(epr3P.tensor, epr3P.offset + 64 + d,
                                     [[96, 128], [4, NCHUNK], [0, 8]])
                            in1 = AP(e_pmS.tensor, e_pmS.offset,
                                     [[64, 128], [8, NCHUNK], [1, 8]])
                            nc.vector.tensor_tensor(
                                out=outap, in0=in0, in1=in1, op=MULT
                            )

                    # num/den and R via block-diagonal matmuls
                    ndP = psA.tile([64, 72], F32, name="ndP", tag="psA")
                    RP = psA.tile([24, 64], F32, name="RP", tag="psA")
                    for k in range(NCHUNK):
                        nc.tensor.matmul(
                            out=ndP[:, :],
                            lhsT=E64S[:, k * 64:(k + 1) * 64],
                            rhs=uniS[:, k * 96:k * 96 + 72],
                            start=(k == 0), stop=False, skip_group_check=True,
                        )
                        nc.tensor.matmul(
                            out=RP[:, k * 8:k * 8 + 8],
                            lhsT=uniS[:, k * 96 + 72:k * 96 + 96],
                            rhs=E16S[:, :],
                            start=True, stop=True, skip_group_check=True,
                        )
                    RS = mw2.tile([24, 64], BF16, name="RS")
                    with nc.allow_low_precision(reason="bf16 staging"):
                        nc.scalar.copy(out=RS[:, :], in_=RP[:, :])
                    # value-side p_r contribution, accumulated into num
                    nc.tensor.matmul(
                        out=ndP[:, 0:64],
                        lhsT=RS[:, :], rhs=W2VES[:, :],
                        start=False, stop=True, skip_group_check=True,
                    )

                    recipS = mw2.tile([64, 8], F32, name="recipS")
                    with nc.allow_low_precision(reason="f32r bitcast, same width"):
                        nc.vector.reciprocal(
                            out=_r(recipS[:, :]), in_=ndP[:, 64:72]
                        )
                    if b % 2 == 0:
                        outS2 = mw2.tile([64, 128], F32, name="outS2", tag="o2")
                    nc.vector.tensor_tensor(
                        out=_r(outS2[:, (b % 2) * 64:(b % 2) * 64 + 64]),
                        in0=ndP[:, 0:64],
                        in1=AP(recipS.tensor, recipS.offset,
                               [[8, 64], [0, 8], [1, 8]]),
                        op=MULT,
                    )
                    if b % 2 == 1:
                        dst = AP(
                            d_out.ap().tensor, (b - 1) * PTS_BLK * C,
                            [[C, 64], [C * 64, 2], [1, C]],
                        )
                        src = AP(
                            outS2.tensor, outS2.offset,
                            [[128, 64], [64, 2], [1, 64]],
                        )
                        nc.sync.dma_start(out=dst, in_=src)

    nc.compile()
    return nc


def kernel(**inputs):
    from concourse.bass_utils import run_bass_kernel_spmd

    shared, per_core = _host_prep(inputs)

    if "nc" not in _CACHED:
        _CACHED["nc"] = _build_program()
    nc = _CACHED["nc"]

    in_maps = []
    for c in range(NCORES):
        m = dict(shared)
        m.update(per_core[c])
        in_maps.append(m)

    res = run_bass_kernel_spmd(nc, in_maps, core_ids=list(range(NCORES)))
    out = np.empty((N, C), np.float32)
    for c in range(NCORES):
        out[c * NPC:(c + 1) * NPC] = res.results[c]["out"][:NPC]
    return out
